# revision 20
# baseline (speedup 1.0000x reference)
"""Trainium2 Bass kernel for nn_CrossAttention (B=4, T=2048, 1024 dims, 16 heads).

Sharding: 8 cores = 4 batches x 2 head-groups (8 heads each). Host sums the two
head-group partials per batch and adds bo; 1/sqrt(D) folded into Wq/bq.

Per core (bf16 matmuls, fp32 PSUM):
  phase 1: kT/v/qT projections (kT,qT: [512,2048] head-dim-major; v: [2048,512]
           with a ones column per head for the softmax denominator).
  phase 2: heads processed in PAIRS (even head A -> kT rows 0-63 of its h-tile,
           odd head B -> rows 64-127). Per tc-tile i: scores for A and B issue
           back-to-back into different PE row groups (concurrent execution,
           LDWEIGHTS pull-ahead), exp(A) feeds PV_A immediately, exp(B) goes to
           a buffered probs ring; PV_B is replayed as filler work inside the
           next pair's exp-bound window. Softmax normalization = ones-column
           denominator, approx-reciprocal, DRAM-bounce partition broadcast.
  phase 3: out_partial = attnT^T @ Wo -> DRAM; chunk-0's projection is filler
           work inside chunk-1's attention, chunk-1's is the tail.
Filler micro-ops (leftover qT projection, PV_B replay, output projection) are
drained a few per iteration to fill the TensorEngine's slack under the
ScalarE(exp) roofline and keep HAM warm.
"""
import numpy as np
import concourse.bacc as bacc
import concourse.mybir as mybir
from concourse.tile import TileContext
from concourse.bass_utils import run_bass_kernel_spmd

N_CORES = 8
P = 128
F = 1024            # query/context feature dim
KF = F // P         # 8 f-tiles
HD = 512            # head-group hidden dim (8 heads x 64)
HT = HD // P        # 4 h-tiles
TQ = TC = 2048
NH = 8              # heads per core
NPAIR = NH // 2
D = 64
CH = 1024           # tq chunk for phase 2/3
NCH = TQ // CH      # 2
TCT = TC // P       # 16 tc tiles
F32 = mybir.dt.float32
F32R = mybir.dt.float32r
BF16 = mybir.dt.bfloat16
PH2_DT = BF16       # qT/kT/vv/probs dtype
PH1_DT = BF16       # inputs/weights dtype (host pre-casts)
DRAIN = 3           # filler micro-ops per phase-2 iteration

_NC_CACHE = None


class FillerQueue:
    """FIFO of micro-op generators; drain() emits up to k instructions."""

    def __init__(self):
        self.gens = []

    def push(self, gen):
        self.gens.append(gen)

    def drain(self, k):
        while k > 0 and self.gens:
            try:
                next(self.gens[0])
                k -= 1
            except StopIteration:
                self.gens.pop(0)

    def drain_all(self):
        while self.gens:
            for _ in self.gens.pop(0):
                pass


def build_kernel(debug=False):
    nc = bacc.Bacc("TRN2", target_bir_lowering=False, debug=False, num_devices=N_CORES)

    qt_d = nc.dram_tensor("qt", [F, TQ], PH1_DT, kind="ExternalInput")   # query[b].T
    ct_d = nc.dram_tensor("ct", [F, TC], PH1_DT, kind="ExternalInput")   # context[b].T
    wq_d = nc.dram_tensor("wq", [F, HD], PH1_DT, kind="ExternalInput")   # pre-scaled 1/8
    wk_d = nc.dram_tensor("wk", [F, HD], PH1_DT, kind="ExternalInput")
    wv_d = nc.dram_tensor("wv", [F, HD], PH1_DT, kind="ExternalInput")
    wo_d = nc.dram_tensor("wo", [HD, 1024], PH1_DT, kind="ExternalInput")
    bq_d = nc.dram_tensor("bq", [HT, P], F32, kind="ExternalInput")      # pre-scaled 1/8
    bk_d = nc.dram_tensor("bk", [HT, P], F32, kind="ExternalInput")
    bv_d = nc.dram_tensor("bv", [1, HD], PH1_DT, kind="ExternalInput")
    out_d = nc.dram_tensor("out", [TQ, 1024], F32, kind="ExternalOutput")
    scr_d = nc.dram_tensor("scr", [NCH, NH, CH], F32)  # denom bounce for broadcast
    if debug:
        at_dump = nc.dram_tensor("at_dump", [NCH, P, HT, CH], F32, kind="ExternalOutput")

    with TileContext(nc) as tc:
        with tc.tile_pool(name="consts", bufs=1) as consts, \
             tc.tile_pool(name="qkv", bufs=1) as qkv, \
             tc.tile_pool(name="wtsq", bufs=1) as wtsq, \
             tc.tile_pool(name="chunks", bufs=2) as chunks, \
             tc.tile_pool(name="wop", bufs=1) as wop, \
             tc.tile_pool(name="probs", bufs=4) as probs, \
             tc.tile_pool(name="pbB", bufs=16) as pbBp, \
             tc.tile_pool(name="attnsb", bufs=2) as attnsb, \
             tc.tile_pool(name="normA", bufs=2) as normA, \
             tc.tile_pool(name="normB", bufs=2) as normB, \
             tc.tile_pool(name="outsb", bufs=2) as outsb, \
             tc.tile_pool(name="ps_proj", bufs=2, space="PSUM") as ps_proj, \
             tc.tile_pool(name="ps_ssA", bufs=1, space="PSUM") as ps_ssA, \
             tc.tile_pool(name="ps_ssB", bufs=1, space="PSUM") as ps_ssB, \
             tc.tile_pool(name="ps_attn", bufs=1, space="PSUM") as ps_attn:

            ones = consts.tile([1, P], PH1_DT)
            nc.vector.memset(ones, 1.0)
            cc_first = chunks.tile([P, KF, 512], PH1_DT, tag="chunk")
            nc.sync.dma_start(out=cc_first,
                              in_=ct_d.rearrange("(k p) t -> p k t", p=P)[:, :, 0:512])
            bv_sb = consts.tile([1, HD], PH1_DT)
            nc.sync.dma_start(out=bv_sb, in_=bv_d[:, :])
            bq_sb = consts.tile([P, HT], F32)
            nc.sync.dma_start(out=bq_sb, in_=bq_d.rearrange("m p -> p m"))
            bk_sb = consts.tile([P, HT], F32)
            nc.sync.dma_start(out=bk_sb, in_=bk_d.rearrange("m p -> p m"))
            # preload the exp table while ScalarE is otherwise idle
            tblin = consts.tile([1, 1], F32)
            nc.vector.memset(tblin, 0.0)
            tbl = consts.tile([1, 1], F32)
            nc.scalar.activation(tbl, tblin, mybir.ActivationFunctionType.Exp)

            qT = qkv.tile([P, HT, TQ], PH2_DT)          # [h-dim, tq]
            kT = qkv.tile([P, HT, TC], PH2_DT)          # [h-dim, tc]
            vv = qkv.tile([P, TCT, NH, D + 1], PH2_DT)  # per head: [v | ones]
            nc.vector.memset(vv[:, :, :, D:D + 1], 1.0)

            qt_r = qt_d.rearrange("(k p) t -> p k t", p=P)
            ct_r = ct_d.rearrange("(k p) t -> p k t", p=P)

            wq = wtsq.tile([P, KF, HD], PH1_DT)
            nc.scalar.dma_start(out=wq, in_=wq_d.rearrange("(k p) h -> p k h", p=P))
            wo = wop.tile([P, HT, 1024], PH1_DT)
            nc.scalar.dma_start(out=wo, in_=wo_d.rearrange("(m p) o -> p m o", p=P))

            # ---------------- phase 1 prefix: K/V for all tc, Q chunks 0,1 -------
            with tc.tile_pool(name="wtskv", bufs=1) as wtskv:
                wk = wtskv.tile([P, KF, HD], PH1_DT)
                nc.scalar.dma_start(out=wk, in_=wk_d.rearrange("(k p) h -> p k h", p=P))
                wv = wtskv.tile([P, KF, HD], PH1_DT)
                nc.scalar.dma_start(out=wv, in_=wv_d.rearrange("(k p) h -> p k h", p=P))

                for n in range(TC // 512):
                    if n == 0:
                        cc = cc_first
                    else:
                        cc = chunks.tile([P, KF, 512], PH1_DT, tag="chunk")
                        nc.sync.dma_start(out=cc, in_=ct_r[:, :, n * 512:(n + 1) * 512])
                    for m in range(HT):
                        ps = ps_proj.tile([P, 512], F32, tag="ps")
                        for k in range(KF):
                            nc.tensor.matmul(ps, wk[:, k, m * P:(m + 1) * P],
                                             cc[:, k, :],
                                             start=(k == 0), stop=(k == KF - 1))
                        nc.vector.tensor_scalar_add(
                            out=kT[:, m, n * 512:(n + 1) * 512], in0=ps,
                            scalar1=bk_sb[:, m:m + 1])
                    for tl in range(4):
                        i = n * 4 + tl
                        ps = ps_proj.tile([P, 512], F32, tag="ps")
                        for k in range(KF):
                            nc.tensor.matmul(ps, cc[:, k, tl * P:(tl + 1) * P],
                                             wv[:, k, :],
                                             start=(k == 0), stop=False)
                        nc.tensor.matmul(ps, ones[0:1, 0:P], bv_sb[0:1, :],
                                         start=False, stop=True)
                        nc.vector.tensor_copy(
                            out=vv[:, i, :, 0:D],
                            in_=ps.rearrange("p (h d) -> p h d", h=NH))

            def qproj_ops(n, qc):
                """Micro-ops: project query chunk n (tq cols n*512..+512)."""
                for m in range(HT):
                    ps = ps_proj.tile([P, 512], F32, tag="ps")
                    for k in range(KF):
                        nc.tensor.matmul(ps, wq[:, k, m * P:(m + 1) * P], qc[:, k, :],
                                         start=(k == 0), stop=(k == KF - 1))
                        yield
                    nc.vector.tensor_scalar_add(
                        out=qT[:, m, n * 512:(n + 1) * 512], in0=ps,
                        scalar1=bq_sb[:, m:m + 1])
                    yield

            def outproj_ops(c, at):
                """Micro-ops: out[tq chunk c] = attnT^T @ Wo -> DRAM."""
                for t in range(CH // P):
                    ot = outsb.tile([P, 1024], F32)
                    for o in range(2):
                        po = ps_proj.tile([P, 512], F32, tag="ps")
                        for m in range(HT):
                            nc.tensor.matmul(po, at[:, m, t * P:(t + 1) * P],
                                             wo[:, m, o * 512:(o + 1) * 512],
                                             start=(m == 0), stop=(m == HT - 1))
                            yield
                        nc.vector.tensor_copy(out=ot[:, o * 512:(o + 1) * 512], in_=po)
                        yield
                    nc.sync.dma_start(
                        out=out_d[c * CH + t * P:c * CH + (t + 1) * P, :], in_=ot)
                    yield

            def pvB_ops(c, p, hB, pbB_tiles, at):
                """Micro-ops: deferred PV + normalize for the odd head of pair p.

                Two [65, 512] half-accumulations (1 PSUM bank each, from the
                ps_proj pool), halves interleaved so each buffered probs tile
                is released after two consecutive micro-ops.
                """
                pa_h = [ps_proj.tile([D + 1, 512], F32, tag="ps", name=f"pah{half}")
                        for half in range(2)]
                for i in range(TCT):
                    for half in range(2):
                        nc.tensor.matmul(
                            pa_h[half], vv[:, i, hB, :],
                            pbB_tiles[i][:, half * 512:(half + 1) * 512],
                            start=(i == 0), stop=(i == TCT - 1))
                        yield
                for half in range(2):
                    hs = slice(half * 512, (half + 1) * 512)
                    ph = normB.tile([D + 1, 512], F32, tag="pasbh")
                    nc.vector.tensor_copy(out=ph, in_=pa_h[half])
                    yield
                    nc.sync.dma_start(out=scr_d[c, hB:hB + 1, hs], in_=ph[D:D + 1, :])
                    dnh = normB.tile([D, 512], F32, tag="dnh")
                    nc.gpsimd.dma_start(
                        out=dnh, in_=scr_d[c, hB:hB + 1, hs].to_broadcast([D, 512]))
                    yield
                    bch = normB.tile([D, 512], F32, tag="bch")
                    nc.vector.reciprocal_approx_fast(out=bch, in_=dnh)
                    yield
                    tmph = normB.tile([D, 512], PH1_DT, tag="tmph")
                    nc.vector.tensor_mul(out=tmph, in0=ph[0:D, :], in1=bch)
                    yield
                    nc.gpsimd.dma_start(out=at[D:P, p, hs], in_=tmph)
                    yield

            # Q chunks 0,1 (tq 0..1023) emitted directly before phase 2
            for n in range(2):
                qc = chunks.tile([P, KF, 512], PH1_DT, tag="chunk")
                nc.sync.dma_start(out=qc, in_=qt_r[:, :, n * 512:(n + 1) * 512])
                for _ in qproj_ops(n, qc):
                    pass

            # ---------------- phase 2 (+ fillers) ----------------
            fillers = FillerQueue()
            at_prev = None
            pending = None
            for c in range(NCH):
                if pending is not None:
                    flush_pv(pending)
                    pending = None
                if c == 0:
                    qc2 = chunks.tile([P, KF, 512], PH1_DT, tag="chunk")
                    nc.sync.dma_start(out=qc2, in_=qt_r[:, :, 2 * 512:3 * 512])
                    qc3 = chunks.tile([P, KF, 512], PH1_DT, tag="chunk")
                    nc.sync.dma_start(out=qc3, in_=qt_r[:, :, 3 * 512:4 * 512])
                    fillers.push(qproj_ops(2, qc2))
                    fillers.push(qproj_ops(3, qc3))
                else:
                    fillers.push(outproj_ops(c - 1, at_prev))

                at = attnsb.tile([P, HT, CH], PH1_DT, tag="attnsb")

                def flush_pv(pend):
                    # PV_A for the previous iteration (one-iteration S lead keeps
                    # ScalarE fed across pair boundaries); at i==15 also emits
                    # head A normalization and defers head B to the filler queue.
                    (fpbA, fpa, fc, fp, fi, fat, fpbB_tiles) = pend
                    fhA, fhB = 2 * fp, 2 * fp + 1
                    for n in range(CH // 512):
                        ns = slice(n * 512, (n + 1) * 512)
                        nc.tensor.matmul(fpa[:, ns], vv[:, fi, fhA, :], fpbA[:, ns],
                                         start=(fi == 0), stop=(fi == TCT - 1))
                    fillers.drain(DRAIN)
                    if fi == TCT - 1:
                        pa_sb = normA.tile([D + 1, CH], F32, tag="pasb")
                        nc.vector.tensor_copy(out=pa_sb, in_=fpa)
                        nc.sync.dma_start(out=scr_d[fc, fhA:fhA + 1, :],
                                          in_=pa_sb[D:D + 1, :])
                        dn = normA.tile([D, CH], F32, tag="dn")
                        nc.gpsimd.dma_start(
                            out=dn, in_=scr_d[fc, fhA:fhA + 1, :].to_broadcast([D, CH]))
                        bc = normA.tile([D, CH], F32, tag="bc")
                        nc.vector.reciprocal_approx_fast(out=bc, in_=dn)
                        nc.vector.tensor_mul(out=fat[0:D, fp, :],
                                             in0=pa_sb[0:D, :], in1=bc)
                        fillers.push(pvB_ops(fc, fp, fhB, fpbB_tiles, fat))

                for p in range(NPAIR):
                    pa = ps_attn.tile([D + 1, CH], F32)
                    pbB_tiles = []
                    for i in range(TCT):
                        ssA = ps_ssA.tile([P, CH], F32)
                        ssB = ps_ssB.tile([P, CH], F32)
                        for n in range(CH // 512):
                            cs = slice(c * CH + n * 512, c * CH + (n + 1) * 512)
                            ns = slice(n * 512, (n + 1) * 512)
                            nc.tensor.matmul(ssA[:, ns], kT[0:D, p, i * P:(i + 1) * P],
                                             qT[0:D, p, cs], start=True, stop=True)
                            nc.tensor.matmul(ssB[:, ns], kT[D:P, p, i * P:(i + 1) * P],
                                             qT[D:P, p, cs], start=True, stop=True)
                        pbA = probs.tile([P, CH], PH2_DT)
                        nc.scalar.activation(pbA, ssA,
                                             mybir.ActivationFunctionType.Exp)
                        pbB = pbBp.tile([P, CH], PH2_DT, tag="pbB")
                        nc.scalar.activation(pbB, ssB,
                                             mybir.ActivationFunctionType.Exp)
                        pbB_tiles.append(pbB)
                        if pending is not None:
                            flush_pv(pending)
                        pending = (pbA, pa, c, p, i, at, pbB_tiles)
                if debug:
                    atf = attnsb.tile([P, HT, CH], F32, tag="at_dbg")
                    nc.vector.tensor_copy(out=atf, in_=at)
                    nc.sync.dma_start(out=at_dump[c], in_=atf)
                at_prev = at
            if pending is not None:
                flush_pv(pending)
                pending = None
            fillers.drain_all()
            # final chunk's output projection (tail)
            for _ in outproj_ops(NCH - 1, at_prev):
                pass

    nc.compile()
    return nc


def make_in_maps(query, context, Wq, bq, Wk, bk, Wv, bv, Wo, bo):
    import ml_dtypes
    cast1 = (lambda a: np.asarray(a, np.float32)) if PH1_DT == F32R \
        else (lambda a: np.asarray(a, np.float32).astype(ml_dtypes.bfloat16))
    query = np.asarray(query, np.float32)
    context = np.asarray(context, np.float32)
    Wq = np.asarray(Wq, np.float32); bq = np.asarray(bq, np.float32)
    Wk = np.asarray(Wk, np.float32); bk = np.asarray(bk, np.float32)
    Wv = np.asarray(Wv, np.float32); bv = np.asarray(bv, np.float32)
    Wo = np.asarray(Wo, np.float32)

    in_maps = []
    for c in range(N_CORES):
        b, g = c // 2, c % 2
        sl = slice(g * HD, (g + 1) * HD)
        in_maps.append({
            "qt": cast1(np.ascontiguousarray(query[b].T)),
            "ct": cast1(np.ascontiguousarray(context[b].T)),
            "wq": cast1(np.ascontiguousarray(Wq[:, sl] * 0.125)),
            "wk": cast1(np.ascontiguousarray(Wk[:, sl])),
            "wv": cast1(np.ascontiguousarray(Wv[:, sl])),
            "wo": cast1(np.ascontiguousarray(Wo[sl, :])),
            "bq": np.ascontiguousarray((bq[sl] * 0.125).reshape(HT, P)),
            "bk": np.ascontiguousarray(bk[sl].reshape(HT, P)),
            "bv": cast1(bv[sl].reshape(1, HD)),
        })
    return in_maps


def kernel(query, context, Wq, bq, Wk, bk, Wv, bv, Wo, bo):
    global _NC_CACHE
    if _NC_CACHE is None:
        _NC_CACHE = build_kernel()
    nc = _NC_CACHE
    bo = np.asarray(bo, np.float32)

    in_maps = make_in_maps(query, context, Wq, bq, Wk, bk, Wv, bv, Wo, bo)
    res = run_bass_kernel_spmd(nc, in_maps, list(range(N_CORES)))
    out = np.empty((4, TQ, 1024), np.float32)
    for b in range(4):
        out[b] = res.results[2 * b]["out"] + res.results[2 * b + 1]["out"] + bo
    return out


# revision 21
# speedup vs baseline: 1.0532x; 1.0532x over previous
"""Trainium2 Bass kernel for nn_CrossAttention (B=4, T=2048, 1024 dims, 16 heads).

Sharding: 8 cores = 4 batches x 2 head-groups (8 heads each). Host sums the two
head-group partials per batch and adds bo; 1/sqrt(D) folded into Wq/bq.

Per core (bf16 matmuls, fp32 PSUM):
  phase 1: kT/v/qT projections (kT,qT: [512,2048] head-dim-major; v: [2048,512]
           with a ones column per head for the softmax denominator).
  phase 2: heads processed in PAIRS (even head A -> kT rows 0-63 of its h-tile,
           odd head B -> rows 64-127). Per tc-tile i: scores for A and B issue
           back-to-back into different PE row groups (concurrent execution,
           LDWEIGHTS pull-ahead), exp(A) feeds PV_A immediately, exp(B) goes to
           a buffered probs ring; PV_B is replayed as filler work inside the
           next pair's exp-bound window. Softmax normalization = ones-column
           denominator, approx-reciprocal, DRAM-bounce partition broadcast.
  phase 3: out_partial = attnT^T @ Wo -> DRAM; chunk-0's projection is filler
           work inside chunk-1's attention, chunk-1's is the tail.
Filler micro-ops (leftover qT projection, PV_B replay, output projection) are
drained a few per iteration to fill the TensorEngine's slack under the
ScalarE(exp) roofline and keep HAM warm.
"""
import numpy as np
import concourse.bacc as bacc
import concourse.mybir as mybir
from concourse.tile import TileContext
from concourse.bass_utils import run_bass_kernel_spmd

N_CORES = 8
P = 128
F = 1024            # query/context feature dim
KF = F // P         # 8 f-tiles
HD = 512            # head-group hidden dim (8 heads x 64)
HT = HD // P        # 4 h-tiles
TQ = TC = 2048
NH = 8              # heads per core
NPAIR = NH // 2
D = 64
CH = 1024           # tq chunk for phase 2/3
NCH = TQ // CH      # 2
TCT = TC // P       # 16 tc tiles
F32 = mybir.dt.float32
F32R = mybir.dt.float32r
BF16 = mybir.dt.bfloat16
PH2_DT = BF16       # qT/kT/vv/probs dtype
PH1_DT = BF16       # inputs/weights dtype (host pre-casts)
DRAIN = 4           # filler micro-ops per phase-2 iteration

_NC_CACHE = None


class FillerQueue:
    """Queue of micro-op generators; drain() emits up to k instructions.

    PV_B replay generators are priority-inserted near the front (so buffered
    probs slots recycle in time for the next pair's exps) but never interrupt
    a mid-flight generator: an interrupted generator's PSUM slots could only
    be released by ops behind the interrupter, deadlocking the in-order PE.
    """

    def __init__(self):
        self.gens = []  # [started, is_pv, gen]

    def push(self, gen, pv=False):
        if pv:
            idx = 1 if (self.gens and self.gens[0][0]) else 0
            while idx < len(self.gens) and self.gens[idx][1]:
                idx += 1
            self.gens.insert(idx, [False, True, gen])
        else:
            self.gens.append([False, False, gen])

    def drain(self, k):
        while k > 0 and self.gens:
            head = self.gens[0]
            try:
                head[0] = True
                next(head[2])
                k -= 1
            except StopIteration:
                self.gens.pop(0)

    def drain_all(self):
        while self.gens:
            for _ in self.gens.pop(0)[2]:
                pass


def build_kernel(debug=False):
    nc = bacc.Bacc("TRN2", target_bir_lowering=False, debug=False, num_devices=N_CORES)

    qt_d = nc.dram_tensor("qt", [F, TQ], PH1_DT, kind="ExternalInput")   # query[b].T
    ct_d = nc.dram_tensor("ct", [F, TC], PH1_DT, kind="ExternalInput")   # context[b].T
    wq_d = nc.dram_tensor("wq", [F, HD], PH1_DT, kind="ExternalInput")   # pre-scaled 1/8
    wk_d = nc.dram_tensor("wk", [F, HD], PH1_DT, kind="ExternalInput")
    wv_d = nc.dram_tensor("wv", [F, HD], PH1_DT, kind="ExternalInput")
    wo_d = nc.dram_tensor("wo", [HD, 1024], PH1_DT, kind="ExternalInput")
    bq_d = nc.dram_tensor("bq", [HT, P], F32, kind="ExternalInput")      # pre-scaled 1/8
    bk_d = nc.dram_tensor("bk", [HT, P], F32, kind="ExternalInput")
    bv_d = nc.dram_tensor("bv", [1, HD], PH1_DT, kind="ExternalInput")
    out_d = nc.dram_tensor("out", [TQ, 1024], F32, kind="ExternalOutput")
    scr_d = nc.dram_tensor("scr", [NCH, NH, CH], F32)  # denom bounce for broadcast
    if debug:
        at_dump = nc.dram_tensor("at_dump", [NCH, P, HT, CH], F32, kind="ExternalOutput")

    with TileContext(nc) as tc:
        with tc.tile_pool(name="consts", bufs=1) as consts, \
             tc.tile_pool(name="qkv", bufs=1) as qkv, \
             tc.tile_pool(name="wtsq", bufs=1) as wtsq, \
             tc.tile_pool(name="chunks", bufs=2) as chunks, \
             tc.tile_pool(name="wop", bufs=1) as wop, \
             tc.tile_pool(name="probs", bufs=4) as probs, \
             tc.tile_pool(name="pbB", bufs=20) as pbBp, \
             tc.tile_pool(name="attnsb", bufs=2) as attnsb, \
             tc.tile_pool(name="normA", bufs=2) as normA, \
             tc.tile_pool(name="normB", bufs=2) as normB, \
             tc.tile_pool(name="outsb", bufs=2) as outsb, \
             tc.tile_pool(name="ps_proj", bufs=2, space="PSUM") as ps_proj, \
             tc.tile_pool(name="ps_ssA", bufs=1, space="PSUM") as ps_ssA, \
             tc.tile_pool(name="ps_ssB", bufs=1, space="PSUM") as ps_ssB, \
             tc.tile_pool(name="ps_attn", bufs=1, space="PSUM") as ps_attn:

            ones = consts.tile([1, P], PH1_DT)
            nc.vector.memset(ones, 1.0)
            cc_first = chunks.tile([P, KF, 512], PH1_DT, tag="chunk")
            nc.sync.dma_start(out=cc_first,
                              in_=ct_d.rearrange("(k p) t -> p k t", p=P)[:, :, 0:512])
            bv_sb = consts.tile([1, HD], PH1_DT)
            nc.sync.dma_start(out=bv_sb, in_=bv_d[:, :])
            bq_sb = consts.tile([P, HT], F32)
            nc.sync.dma_start(out=bq_sb, in_=bq_d.rearrange("m p -> p m"))
            bk_sb = consts.tile([P, HT], F32)
            nc.sync.dma_start(out=bk_sb, in_=bk_d.rearrange("m p -> p m"))
            # preload the exp table while ScalarE is otherwise idle
            tblin = consts.tile([1, 1], F32)
            nc.vector.memset(tblin, 0.0)
            tbl = consts.tile([1, 1], F32)
            nc.scalar.activation(tbl, tblin, mybir.ActivationFunctionType.Exp)

            qT = qkv.tile([P, HT, TQ], PH2_DT)          # [h-dim, tq]
            kT = qkv.tile([P, HT, TC], PH2_DT)          # [h-dim, tc]
            vv = qkv.tile([P, TCT, NH, D + 1], PH2_DT)  # per head: [v | ones]
            nc.vector.memset(vv[:, :, :, D:D + 1], 1.0)

            qt_r = qt_d.rearrange("(k p) t -> p k t", p=P)
            ct_r = ct_d.rearrange("(k p) t -> p k t", p=P)

            wq = wtsq.tile([P, KF, HD], PH1_DT)
            nc.scalar.dma_start(out=wq, in_=wq_d.rearrange("(k p) h -> p k h", p=P))
            wo = wop.tile([P, HT, 1024], PH1_DT)
            nc.scalar.dma_start(out=wo, in_=wo_d.rearrange("(m p) o -> p m o", p=P))

            # ---------------- phase 1 prefix: K/V for all tc, Q chunks 0,1 -------
            with tc.tile_pool(name="wtskv", bufs=1) as wtskv:
                wk = wtskv.tile([P, KF, HD], PH1_DT)
                nc.scalar.dma_start(out=wk, in_=wk_d.rearrange("(k p) h -> p k h", p=P))
                wv = wtskv.tile([P, KF, HD], PH1_DT)
                nc.scalar.dma_start(out=wv, in_=wv_d.rearrange("(k p) h -> p k h", p=P))

                for n in range(TC // 512):
                    if n == 0:
                        cc = cc_first
                    else:
                        cc = chunks.tile([P, KF, 512], PH1_DT, tag="chunk")
                        nc.sync.dma_start(out=cc, in_=ct_r[:, :, n * 512:(n + 1) * 512])
                    for m in range(HT):
                        ps = ps_proj.tile([P, 512], F32, tag="ps")
                        for k in range(KF):
                            nc.tensor.matmul(ps, wk[:, k, m * P:(m + 1) * P],
                                             cc[:, k, :],
                                             start=(k == 0), stop=(k == KF - 1))
                        nc.vector.tensor_scalar_add(
                            out=kT[:, m, n * 512:(n + 1) * 512], in0=ps,
                            scalar1=bk_sb[:, m:m + 1])
                    for tl in range(4):
                        i = n * 4 + tl
                        ps = ps_proj.tile([P, 512], F32, tag="ps")
                        for k in range(KF):
                            nc.tensor.matmul(ps, cc[:, k, tl * P:(tl + 1) * P],
                                             wv[:, k, :],
                                             start=(k == 0), stop=False)
                        nc.tensor.matmul(ps, ones[0:1, 0:P], bv_sb[0:1, :],
                                         start=False, stop=True)
                        nc.vector.tensor_copy(
                            out=vv[:, i, :, 0:D],
                            in_=ps.rearrange("p (h d) -> p h d", h=NH))

            def qproj_ops(n, qc):
                """Micro-ops: project query chunk n (tq cols n*512..+512)."""
                for m in range(HT):
                    ps = ps_proj.tile([P, 512], F32, tag="ps")
                    for k in range(KF):
                        nc.tensor.matmul(ps, wq[:, k, m * P:(m + 1) * P], qc[:, k, :],
                                         start=(k == 0), stop=(k == KF - 1))
                        yield
                    nc.vector.tensor_scalar_add(
                        out=qT[:, m, n * 512:(n + 1) * 512], in0=ps,
                        scalar1=bq_sb[:, m:m + 1])
                    yield

            def outproj_ops(c, at):
                """Micro-ops: out[tq chunk c] = attnT^T @ Wo -> DRAM."""
                for t in range(CH // P):
                    ot = outsb.tile([P, 1024], F32)
                    for o in range(2):
                        po = ps_proj.tile([P, 512], F32, tag="ps")
                        for m in range(HT):
                            nc.tensor.matmul(po, at[:, m, t * P:(t + 1) * P],
                                             wo[:, m, o * 512:(o + 1) * 512],
                                             start=(m == 0), stop=(m == HT - 1))
                            yield
                        nc.vector.tensor_copy(out=ot[:, o * 512:(o + 1) * 512], in_=po)
                        yield
                    nc.sync.dma_start(
                        out=out_d[c * CH + t * P:c * CH + (t + 1) * P, :], in_=ot)
                    yield

            def pvB_ops(c, p, hB, pbB_tiles, at):
                """Micro-ops: deferred PV + normalize for the odd head of pair p.

                Two [65, 512] half-accumulations (1 PSUM bank each, from the
                ps_proj pool), halves interleaved so each buffered probs tile
                is released after two consecutive micro-ops.
                """
                pa_h = [ps_proj.tile([D + 1, 512], F32, tag="ps", name=f"pah{half}")
                        for half in range(2)]
                for i in range(TCT):
                    for half in range(2):
                        nc.tensor.matmul(
                            pa_h[half], vv[:, i, hB, :],
                            pbB_tiles[i][:, half * 512:(half + 1) * 512],
                            start=(i == 0), stop=(i == TCT - 1))
                        yield
                for half in range(2):
                    hs = slice(half * 512, (half + 1) * 512)
                    ph = normB.tile([D + 1, 512], F32, tag="pasbh")
                    nc.vector.tensor_copy(out=ph, in_=pa_h[half])
                    yield
                    nc.sync.dma_start(out=scr_d[c, hB:hB + 1, hs], in_=ph[D:D + 1, :])
                    dnh = normB.tile([D, 512], F32, tag="dnh")
                    nc.gpsimd.dma_start(
                        out=dnh, in_=scr_d[c, hB:hB + 1, hs].to_broadcast([D, 512]))
                    yield
                    bch = normB.tile([D, 512], F32, tag="bch")
                    nc.vector.reciprocal_approx_fast(out=bch, in_=dnh)
                    yield
                    tmph = normB.tile([D, 512], PH1_DT, tag="tmph")
                    nc.vector.tensor_mul(out=tmph, in0=ph[0:D, :], in1=bch)
                    yield
                    nc.gpsimd.dma_start(out=at[D:P, p, hs], in_=tmph)
                    yield

            # Q chunks 0,1 (tq 0..1023) emitted directly before phase 2
            for n in range(2):
                qc = chunks.tile([P, KF, 512], PH1_DT, tag="chunk")
                nc.sync.dma_start(out=qc, in_=qt_r[:, :, n * 512:(n + 1) * 512])
                for _ in qproj_ops(n, qc):
                    pass

            # ---------------- phase 2 (+ fillers) ----------------
            fillers = FillerQueue()
            at_prev = None
            pending = None
            for c in range(NCH):
                if pending is not None:
                    flush_pv(pending)
                    pending = None
                if c == 0:
                    qc2 = chunks.tile([P, KF, 512], PH1_DT, tag="chunk")
                    nc.sync.dma_start(out=qc2, in_=qt_r[:, :, 2 * 512:3 * 512])
                    qc3 = chunks.tile([P, KF, 512], PH1_DT, tag="chunk")
                    nc.sync.dma_start(out=qc3, in_=qt_r[:, :, 3 * 512:4 * 512])
                    fillers.push(qproj_ops(2, qc2))
                    fillers.push(qproj_ops(3, qc3))
                else:
                    fillers.push(outproj_ops(c - 1, at_prev))

                at = attnsb.tile([P, HT, CH], PH1_DT, tag="attnsb")

                def flush_pv(pend):
                    # PV_A for the previous iteration (one-iteration S lead keeps
                    # ScalarE fed across pair boundaries); at i==15 also emits
                    # head A normalization and defers head B to the filler queue.
                    (fpbA, fpa, fc, fp, fi, fat, fpbB_tiles) = pend
                    fhA, fhB = 2 * fp, 2 * fp + 1
                    for n in range(CH // 512):
                        ns = slice(n * 512, (n + 1) * 512)
                        nc.tensor.matmul(fpa[:, ns], vv[:, fi, fhA, :], fpbA[:, ns],
                                         start=(fi == 0), stop=(fi == TCT - 1))
                    fillers.drain(DRAIN)
                    if fi == TCT - 1:
                        pa_sb = normA.tile([D + 1, CH], F32, tag="pasb")
                        nc.vector.tensor_copy(out=pa_sb, in_=fpa)
                        nc.sync.dma_start(out=scr_d[fc, fhA:fhA + 1, :],
                                          in_=pa_sb[D:D + 1, :])
                        dn = normA.tile([D, CH], F32, tag="dn")
                        nc.gpsimd.dma_start(
                            out=dn, in_=scr_d[fc, fhA:fhA + 1, :].to_broadcast([D, CH]))
                        bc = normA.tile([D, CH], F32, tag="bc")
                        nc.vector.reciprocal_approx_fast(out=bc, in_=dn)
                        nc.vector.tensor_mul(out=fat[0:D, fp, :],
                                             in0=pa_sb[0:D, :], in1=bc)
                        fillers.push(pvB_ops(fc, fp, fhB, fpbB_tiles, fat), pv=True)

                for p in range(NPAIR):
                    pa = ps_attn.tile([D + 1, CH], F32)
                    pbB_tiles = []
                    for i in range(TCT):
                        ssA = ps_ssA.tile([P, CH], F32)
                        ssB = ps_ssB.tile([P, CH], F32)
                        for n in range(CH // 512):
                            cs = slice(c * CH + n * 512, c * CH + (n + 1) * 512)
                            ns = slice(n * 512, (n + 1) * 512)
                            nc.tensor.matmul(ssA[:, ns], kT[0:D, p, i * P:(i + 1) * P],
                                             qT[0:D, p, cs], start=True, stop=True)
                            nc.tensor.matmul(ssB[:, ns], kT[D:P, p, i * P:(i + 1) * P],
                                             qT[D:P, p, cs], start=True, stop=True)
                        pbA = probs.tile([P, CH], PH2_DT)
                        nc.scalar.activation(pbA, ssA,
                                             mybir.ActivationFunctionType.Exp)
                        pbB = pbBp.tile([P, CH], PH2_DT, tag="pbB")
                        nc.scalar.activation(pbB, ssB,
                                             mybir.ActivationFunctionType.Exp)
                        pbB_tiles.append(pbB)
                        if pending is not None:
                            flush_pv(pending)
                        pending = (pbA, pa, c, p, i, at, pbB_tiles)
                if debug:
                    atf = attnsb.tile([P, HT, CH], F32, tag="at_dbg")
                    nc.vector.tensor_copy(out=atf, in_=at)
                    nc.sync.dma_start(out=at_dump[c], in_=atf)
                at_prev = at
            if pending is not None:
                flush_pv(pending)
                pending = None
            fillers.drain_all()
            # final chunk's output projection (tail)
            for _ in outproj_ops(NCH - 1, at_prev):
                pass

    nc.compile()
    return nc


def make_in_maps(query, context, Wq, bq, Wk, bk, Wv, bv, Wo, bo):
    import ml_dtypes
    cast1 = (lambda a: np.asarray(a, np.float32)) if PH1_DT == F32R \
        else (lambda a: np.asarray(a, np.float32).astype(ml_dtypes.bfloat16))
    query = np.asarray(query, np.float32)
    context = np.asarray(context, np.float32)
    Wq = np.asarray(Wq, np.float32); bq = np.asarray(bq, np.float32)
    Wk = np.asarray(Wk, np.float32); bk = np.asarray(bk, np.float32)
    Wv = np.asarray(Wv, np.float32); bv = np.asarray(bv, np.float32)
    Wo = np.asarray(Wo, np.float32)

    in_maps = []
    for c in range(N_CORES):
        b, g = c // 2, c % 2
        sl = slice(g * HD, (g + 1) * HD)
        in_maps.append({
            "qt": cast1(np.ascontiguousarray(query[b].T)),
            "ct": cast1(np.ascontiguousarray(context[b].T)),
            "wq": cast1(np.ascontiguousarray(Wq[:, sl] * 0.125)),
            "wk": cast1(np.ascontiguousarray(Wk[:, sl])),
            "wv": cast1(np.ascontiguousarray(Wv[:, sl])),
            "wo": cast1(np.ascontiguousarray(Wo[sl, :])),
            "bq": np.ascontiguousarray((bq[sl] * 0.125).reshape(HT, P)),
            "bk": np.ascontiguousarray(bk[sl].reshape(HT, P)),
            "bv": cast1(bv[sl].reshape(1, HD)),
        })
    return in_maps


def kernel(query, context, Wq, bq, Wk, bk, Wv, bv, Wo, bo):
    global _NC_CACHE
    if _NC_CACHE is None:
        _NC_CACHE = build_kernel()
    nc = _NC_CACHE
    bo = np.asarray(bo, np.float32)

    in_maps = make_in_maps(query, context, Wq, bq, Wk, bk, Wv, bv, Wo, bo)
    res = run_bass_kernel_spmd(nc, in_maps, list(range(N_CORES)))
    out = np.empty((4, TQ, 1024), np.float32)
    for b in range(4):
        out[b] = res.results[2 * b]["out"] + res.results[2 * b + 1]["out"] + bo
    return out


# revision 22
# speedup vs baseline: 1.1200x; 1.0635x over previous
"""Trainium2 Bass kernel for nn_CrossAttention (B=4, T=2048, 1024 dims, 16 heads).

Sharding: 8 cores = 4 batches x 2 head-groups (8 heads each). Host sums the two
head-group partials per batch and adds bo; 1/sqrt(D) folded into Wq/bq.

Per core (bf16 matmuls, fp32 PSUM):
  phase 1: kT/v/qT projections (kT,qT: [512,2048] head-dim-major; v: [2048,512]
           with a ones column per head for the softmax denominator).
  phase 2: heads processed in PAIRS (even head A -> kT rows 0-63 of its h-tile,
           odd head B -> rows 64-127). Per tc-tile i: scores for A and B issue
           back-to-back into different PE row groups (concurrent execution,
           LDWEIGHTS pull-ahead), exp(A) feeds PV_A immediately, exp(B) goes to
           a buffered probs ring; PV_B is replayed as filler work inside the
           next pair's exp-bound window. Softmax normalization = ones-column
           denominator, approx-reciprocal, DRAM-bounce partition broadcast.
  phase 3: out_partial = attnT^T @ Wo -> DRAM; chunk-0's projection is filler
           work inside chunk-1's attention, chunk-1's is the tail.
Filler micro-ops (leftover qT projection, PV_B replay, output projection) are
drained a few per iteration to fill the TensorEngine's slack under the
ScalarE(exp) roofline and keep HAM warm.
"""
import numpy as np
import concourse.bacc as bacc
import concourse.mybir as mybir
from concourse.tile import TileContext
from concourse.bass_utils import run_bass_kernel_spmd

N_CORES = 8
P = 128
F = 1024            # query/context feature dim
KF = F // P         # 8 f-tiles
HD = 512            # head-group hidden dim (8 heads x 64)
HT = HD // P        # 4 h-tiles
TQ = TC = 2048
NH = 8              # heads per core
NPAIR = NH // 2
D = 64
CH = 1024           # tq chunk for phase 2/3
NCH = TQ // CH      # 2
TCT = TC // P       # 16 tc tiles
F32 = mybir.dt.float32
F32R = mybir.dt.float32r
BF16 = mybir.dt.bfloat16
PH2_DT = BF16       # qT/kT/vv/probs dtype
PH1_DT = BF16       # inputs/weights dtype (host pre-casts)
DRAIN = 2           # filler micro-ops per phase-2 iteration

_NC_CACHE = None


class FillerQueue:
    """Queue of micro-op generators; drain() emits up to k instructions.

    PV_B replay generators are priority-inserted near the front (so buffered
    probs slots recycle in time for the next pair's exps) but never interrupt
    a mid-flight generator: an interrupted generator's PSUM slots could only
    be released by ops behind the interrupter, deadlocking the in-order PE.
    """

    def __init__(self):
        self.gens = []  # [started, is_pv, gen]

    def push(self, gen, pv=False):
        if pv:
            idx = 1 if (self.gens and self.gens[0][0]) else 0
            while idx < len(self.gens) and self.gens[idx][1]:
                idx += 1
            self.gens.insert(idx, [False, True, gen])
        else:
            self.gens.append([False, False, gen])

    def drain(self, k):
        while k > 0 and self.gens:
            head = self.gens[0]
            try:
                head[0] = True
                next(head[2])
                k -= 1
            except StopIteration:
                self.gens.pop(0)

    def drain_all(self):
        while self.gens:
            for _ in self.gens.pop(0)[2]:
                pass


def build_kernel(debug=False):
    nc = bacc.Bacc("TRN2", target_bir_lowering=False, debug=False, num_devices=N_CORES)

    qt_d = nc.dram_tensor("qt", [F, TQ], PH1_DT, kind="ExternalInput")   # query[b].T
    ct_d = nc.dram_tensor("ct", [F, TC], PH1_DT, kind="ExternalInput")   # context[b].T
    wq_d = nc.dram_tensor("wq", [F, HD], PH1_DT, kind="ExternalInput")   # pre-scaled 1/8
    wk_d = nc.dram_tensor("wk", [F, HD], PH1_DT, kind="ExternalInput")
    wv_d = nc.dram_tensor("wv", [F, HD], PH1_DT, kind="ExternalInput")
    wo_d = nc.dram_tensor("wo", [HD, 1024], PH1_DT, kind="ExternalInput")
    bq_d = nc.dram_tensor("bq", [HT, P], F32, kind="ExternalInput")      # pre-scaled 1/8
    bk_d = nc.dram_tensor("bk", [HT, P], F32, kind="ExternalInput")
    bv_d = nc.dram_tensor("bv", [1, HD], PH1_DT, kind="ExternalInput")
    out_d = nc.dram_tensor("out", [TQ, 1024], F32, kind="ExternalOutput")
    scr_d = nc.dram_tensor("scr", [NCH, NH, CH], F32)  # denom bounce for broadcast
    if debug:
        at_dump = nc.dram_tensor("at_dump", [NCH, P, HT, CH], F32, kind="ExternalOutput")

    with TileContext(nc) as tc:
        with tc.tile_pool(name="consts", bufs=1) as consts, \
             tc.tile_pool(name="qkv", bufs=1) as qkv, \
             tc.tile_pool(name="wtsq", bufs=1) as wtsq, \
             tc.tile_pool(name="chunks", bufs=2) as chunks, \
             tc.tile_pool(name="wop", bufs=1) as wop, \
             tc.tile_pool(name="probs", bufs=4) as probs, \
             tc.tile_pool(name="attnsb", bufs=2) as attnsb, \
             tc.tile_pool(name="normB", bufs=2) as normB, \
             tc.tile_pool(name="outsb", bufs=2) as outsb, \
             tc.tile_pool(name="ps_proj", bufs=2, space="PSUM") as ps_proj, \
             tc.tile_pool(name="ps_ss", bufs=2, space="PSUM") as ps_ss, \
             tc.tile_pool(name="ps_attn", bufs=1, space="PSUM") as ps_attn:

            ones = consts.tile([1, P], PH1_DT)
            nc.vector.memset(ones, 1.0)
            cc_first = chunks.tile([P, KF, 512], PH1_DT, tag="chunk")
            nc.sync.dma_start(out=cc_first,
                              in_=ct_d.rearrange("(k p) t -> p k t", p=P)[:, :, 0:512])
            bv_sb = consts.tile([1, HD], PH1_DT)
            nc.sync.dma_start(out=bv_sb, in_=bv_d[:, :])
            bq_sb = consts.tile([P, HT], F32)
            nc.sync.dma_start(out=bq_sb, in_=bq_d.rearrange("m p -> p m"))
            bk_sb = consts.tile([P, HT], F32)
            nc.sync.dma_start(out=bk_sb, in_=bk_d.rearrange("m p -> p m"))
            # preload the exp table while ScalarE is otherwise idle
            tblin = consts.tile([1, 1], F32)
            nc.vector.memset(tblin, 0.0)
            tbl = consts.tile([1, 1], F32)
            nc.scalar.activation(tbl, tblin, mybir.ActivationFunctionType.Exp)

            qT = qkv.tile([P, HT, TQ], PH2_DT)          # [h-dim, tq]
            kT = qkv.tile([P, HT, TC], PH2_DT)          # [h-dim, tc]
            vv = qkv.tile([P, TCT, NH, D + 1], PH2_DT)  # per head: [v | ones]
            nc.vector.memset(vv[:, :, :, D:D + 1], 1.0)

            qt_r = qt_d.rearrange("(k p) t -> p k t", p=P)
            ct_r = ct_d.rearrange("(k p) t -> p k t", p=P)

            wq = wtsq.tile([P, KF, HD], PH1_DT)
            nc.scalar.dma_start(out=wq, in_=wq_d.rearrange("(k p) h -> p k h", p=P))
            wo = wop.tile([P, HT, 1024], PH1_DT)
            nc.scalar.dma_start(out=wo, in_=wo_d.rearrange("(m p) o -> p m o", p=P))

            # ---------------- phase 1 prefix: K/V for all tc, Q chunks 0,1 -------
            with tc.tile_pool(name="wtskv", bufs=1) as wtskv:
                wk = wtskv.tile([P, KF, HD], PH1_DT)
                nc.scalar.dma_start(out=wk, in_=wk_d.rearrange("(k p) h -> p k h", p=P))
                wv = wtskv.tile([P, KF, HD], PH1_DT)
                nc.scalar.dma_start(out=wv, in_=wv_d.rearrange("(k p) h -> p k h", p=P))

                for n in range(TC // 512):
                    if n == 0:
                        cc = cc_first
                    else:
                        cc = chunks.tile([P, KF, 512], PH1_DT, tag="chunk")
                        nc.sync.dma_start(out=cc, in_=ct_r[:, :, n * 512:(n + 1) * 512])
                    for m in range(HT):
                        ps = ps_proj.tile([P, 512], F32, tag="ps")
                        for k in range(KF):
                            nc.tensor.matmul(ps, wk[:, k, m * P:(m + 1) * P],
                                             cc[:, k, :],
                                             start=(k == 0), stop=(k == KF - 1))
                        nc.vector.tensor_scalar_add(
                            out=kT[:, m, n * 512:(n + 1) * 512], in0=ps,
                            scalar1=bk_sb[:, m:m + 1])
                    for tl in range(4):
                        i = n * 4 + tl
                        ps = ps_proj.tile([P, 512], F32, tag="ps")
                        for k in range(KF):
                            nc.tensor.matmul(ps, cc[:, k, tl * P:(tl + 1) * P],
                                             wv[:, k, :],
                                             start=(k == 0), stop=False)
                        nc.tensor.matmul(ps, ones[0:1, 0:P], bv_sb[0:1, :],
                                         start=False, stop=True)
                        nc.vector.tensor_copy(
                            out=vv[:, i, :, 0:D],
                            in_=ps.rearrange("p (h d) -> p h d", h=NH))

            def qproj_ops(n, qc):
                """Micro-ops: project query chunk n (tq cols n*512..+512)."""
                for m in range(HT):
                    ps = ps_proj.tile([P, 512], F32, tag="ps")
                    for k in range(KF):
                        nc.tensor.matmul(ps, wq[:, k, m * P:(m + 1) * P], qc[:, k, :],
                                         start=(k == 0), stop=(k == KF - 1))
                        yield
                    nc.vector.tensor_scalar_add(
                        out=qT[:, m, n * 512:(n + 1) * 512], in0=ps,
                        scalar1=bq_sb[:, m:m + 1])
                    yield

            def outproj_ops(c, at):
                """Micro-ops: out[tq chunk c] = attnT^T @ Wo -> DRAM."""
                for t in range(CH // P):
                    ot = outsb.tile([P, 1024], F32)
                    for o in range(2):
                        po = ps_proj.tile([P, 512], F32, tag="ps")
                        for m in range(HT):
                            nc.tensor.matmul(po, at[:, m, t * P:(t + 1) * P],
                                             wo[:, m, o * 512:(o + 1) * 512],
                                             start=(m == 0), stop=(m == HT - 1))
                            yield
                        nc.vector.tensor_copy(out=ot[:, o * 512:(o + 1) * 512], in_=po)
                        yield
                    nc.sync.dma_start(
                        out=out_d[c * CH + t * P:c * CH + (t + 1) * P, :], in_=ot)
                    yield

            # Q chunks 0,1 (tq 0..1023) emitted directly before phase 2
            for n in range(2):
                qc = chunks.tile([P, KF, 512], PH1_DT, tag="chunk")
                nc.sync.dma_start(out=qc, in_=qt_r[:, :, n * 512:(n + 1) * 512])
                for _ in qproj_ops(n, qc):
                    pass

            # ---------------- phase 2 (+ fillers) ----------------
            # Heads in pairs (A=even -> kT rows 0-63, B=odd -> rows 64-127),
            # tq processed in 512-wide halves. Both heads' scores for a half
            # land in ONE [128, 1024] PSUM tile consumed by ONE exp, so the
            # two score matmuls share a release event and co-execute in
            # different PE row groups. PV for both heads follows immediately
            # ([65, 512] accumulators, 1 bank each). PV lags scores by one
            # iteration so the PE never waits on the just-issued exp.
            fillers = FillerQueue()
            at_prev = None
            pending = None

            def norm_half(c, h, p, half, pa, at):
                """Normalize one [64, tq-512] attn block of head h into at."""
                hs = slice(half * 512, (half + 1) * 512)
                ph = normB.tile([D + 1, 512], F32, tag="pasbh")
                nc.vector.tensor_copy(out=ph, in_=pa)
                nc.sync.dma_start(out=scr_d[c, h:h + 1, hs], in_=ph[D:D + 1, :])
                dnh = normB.tile([D, 512], F32, tag="dnh")
                nc.gpsimd.dma_start(
                    out=dnh, in_=scr_d[c, h:h + 1, hs].to_broadcast([D, 512]))
                bch = normB.tile([D, 512], F32, tag="bch")
                nc.vector.reciprocal_approx_fast(out=bch, in_=dnh)
                if h % 2 == 0:
                    nc.vector.tensor_mul(out=at[0:D, p, hs], in0=ph[0:D, :], in1=bch)
                else:
                    tmph = normB.tile([D, 512], PH1_DT, tag="tmph")
                    nc.vector.tensor_mul(out=tmph, in0=ph[0:D, :], in1=bch)
                    nc.gpsimd.dma_start(out=at[D:P, p, hs], in_=tmph)

            def flush_pv(pend):
                (fpb, fpaA, fpaB, fc, fp, fhalf, fi, fat) = pend
                nc.tensor.matmul(fpaA, vv[:, fi, 2 * fp, :], fpb[:, 0:512],
                                 start=(fi == 0), stop=(fi == TCT - 1))
                nc.tensor.matmul(fpaB, vv[:, fi, 2 * fp + 1, :], fpb[:, 512:1024],
                                 start=(fi == 0), stop=(fi == TCT - 1))
                fillers.drain(DRAIN)
                if fi == TCT - 1:
                    norm_half(fc, 2 * fp, fp, fhalf, fpaA, fat)
                    norm_half(fc, 2 * fp + 1, fp, fhalf, fpaB, fat)

            for c in range(NCH):
                if c == 0:
                    qc2 = chunks.tile([P, KF, 512], PH1_DT, tag="chunk")
                    nc.sync.dma_start(out=qc2, in_=qt_r[:, :, 2 * 512:3 * 512])
                    qc3 = chunks.tile([P, KF, 512], PH1_DT, tag="chunk")
                    nc.sync.dma_start(out=qc3, in_=qt_r[:, :, 3 * 512:4 * 512])
                    fillers.push(qproj_ops(2, qc2))
                    fillers.push(qproj_ops(3, qc3))
                else:
                    fillers.push(outproj_ops(c - 1, at_prev))

                at = attnsb.tile([P, HT, CH], PH1_DT, tag="attnsb")
                for p in range(NPAIR):
                    for half in range(2):
                        cs = slice(c * CH + half * 512, c * CH + (half + 1) * 512)
                        paA = ps_attn.tile([D + 1, 512], F32, tag="paA")
                        paB = ps_attn.tile([D + 1, 512], F32, tag="paB")
                        for i in range(TCT):
                            ss = ps_ss.tile([P, 1024], F32, tag="ss")
                            nc.tensor.matmul(ss[:, 0:512],
                                             kT[0:D, p, i * P:(i + 1) * P],
                                             qT[0:D, p, cs], start=True, stop=True)
                            nc.tensor.matmul(ss[:, 512:1024],
                                             kT[D:P, p, i * P:(i + 1) * P],
                                             qT[D:P, p, cs], start=True, stop=True)
                            pb = probs.tile([P, 1024], PH2_DT)
                            nc.scalar.activation(pb, ss,
                                                 mybir.ActivationFunctionType.Exp)
                            if pending is not None:
                                flush_pv(pending)
                            pending = (pb, paA, paB, c, p, half, i, at)
                if debug:
                    atf = attnsb.tile([P, HT, CH], F32, tag="at_dbg")
                    nc.vector.tensor_copy(out=atf, in_=at)
                    nc.sync.dma_start(out=at_dump[c], in_=atf)
                at_prev = at
            if pending is not None:
                flush_pv(pending)
                pending = None
            fillers.drain_all()
            # final chunk's output projection (tail)
            for _ in outproj_ops(NCH - 1, at_prev):
                pass

    nc.compile()
    return nc


def make_in_maps(query, context, Wq, bq, Wk, bk, Wv, bv, Wo, bo):
    import ml_dtypes
    cast1 = (lambda a: np.asarray(a, np.float32)) if PH1_DT == F32R \
        else (lambda a: np.asarray(a, np.float32).astype(ml_dtypes.bfloat16))
    query = np.asarray(query, np.float32)
    context = np.asarray(context, np.float32)
    Wq = np.asarray(Wq, np.float32); bq = np.asarray(bq, np.float32)
    Wk = np.asarray(Wk, np.float32); bk = np.asarray(bk, np.float32)
    Wv = np.asarray(Wv, np.float32); bv = np.asarray(bv, np.float32)
    Wo = np.asarray(Wo, np.float32)

    in_maps = []
    for c in range(N_CORES):
        b, g = c // 2, c % 2
        sl = slice(g * HD, (g + 1) * HD)
        in_maps.append({
            "qt": cast1(np.ascontiguousarray(query[b].T)),
            "ct": cast1(np.ascontiguousarray(context[b].T)),
            "wq": cast1(np.ascontiguousarray(Wq[:, sl] * 0.125)),
            "wk": cast1(np.ascontiguousarray(Wk[:, sl])),
            "wv": cast1(np.ascontiguousarray(Wv[:, sl])),
            "wo": cast1(np.ascontiguousarray(Wo[sl, :])),
            "bq": np.ascontiguousarray((bq[sl] * 0.125).reshape(HT, P)),
            "bk": np.ascontiguousarray(bk[sl].reshape(HT, P)),
            "bv": cast1(bv[sl].reshape(1, HD)),
        })
    return in_maps


def kernel(query, context, Wq, bq, Wk, bk, Wv, bv, Wo, bo):
    global _NC_CACHE
    if _NC_CACHE is None:
        _NC_CACHE = build_kernel()
    nc = _NC_CACHE
    bo = np.asarray(bo, np.float32)

    in_maps = make_in_maps(query, context, Wq, bq, Wk, bk, Wv, bv, Wo, bo)
    res = run_bass_kernel_spmd(nc, in_maps, list(range(N_CORES)))
    out = np.empty((4, TQ, 1024), np.float32)
    for b in range(4):
        out[b] = res.results[2 * b]["out"] + res.results[2 * b + 1]["out"] + bo
    return out
